# revision 1
# baseline (speedup 1.0000x reference)
"""Trainium2 Bass kernel for nn_AntiAttentionSegment (sparse segment attention).

Sharding: 8 cores, segment-expert. Core c handles global segments {2c, 2c+1}
for all 8 batch elements. The only cross-core communication is an AllGather of
the per-segment lord vectors ([16, 8, 1024] fp32 = 512KB) for the tiny global
lord attention, which every core then computes redundantly for its own rows.

Layout strategy: "transposed" activations (feature dim on SBUF partitions) so
every matmul contracts over partitions with the weight (pre-transposed on the
host) as the stationary operand. Scores are computed transposed (S^T[tk, tq])
so softmax denominators come from a ones-vector matmul and the PV product needs
no transposes. All large matmuls run as float32r (TF32-rate = bf16-rate for
moving dim >= 256); the probs @ V product runs in bf16.
"""

import numpy as np

import concourse.bass as bass
from concourse import bacc, mybir
from concourse.tile import TileContext

F32R = mybir.dt.float32r
F32 = mybir.dt.float32
BF16 = mybir.dt.bfloat16
AF = mybir.ActivationFunctionType
ALU = mybir.AluOpType

B = 8          # batch
SEG = 16       # total segments
SL = 2         # segments per core
T = 257        # tokens per segment
TOK = SL * T   # 514 tokens per core
H = 1024
NH = 16
DH = 64
KT = 8         # H / 128 partition tiles
N_CORES = 8
L = 1 + SEG * T
SCALE = 0.125  # 1/sqrt(DH)
LN_EPS = 1e-12

TKT = [(0, 128), (128, 128), (256, 1)]  # (t0, tsz) tk tiles of 257
TP = 258   # padded tile width (fp32r needs an even moving dim)
DBG = False


def build_program():
    nc = bacc.Bacc(target_bir_lowering=False)

    xT = nc.declare_dram_parameter("xT", [B, KT, 128, TOK], BF16, isOutput=False)
    xnat = nc.declare_dram_parameter("xnat", [B, TOK, H], F32, isOutput=False)
    h0T = nc.declare_dram_parameter("h0T", [KT, 128, B], BF16, isOutput=False)
    h0nat = nc.declare_dram_parameter("h0nat", [B, H], F32, isOutput=False)
    qwT = nc.declare_dram_parameter("qwT", [SL, KT, 128, H], BF16, isOutput=False)
    kwT = nc.declare_dram_parameter("kwT", [SL, KT, 128, H], BF16, isOutput=False)
    vwT = nc.declare_dram_parameter("vwT", [SL, KT, 128, H], BF16, isOutput=False)
    blwT = nc.declare_dram_parameter("blwT", [KT, 128, H], BF16, isOutput=False)
    blb = nc.declare_dram_parameter("blb", [H], F32, isOutput=False)
    lqwT = nc.declare_dram_parameter("lqwT", [KT, 128, H], BF16, isOutput=False)
    lkwT = nc.declare_dram_parameter("lkwT", [KT, 128, H], BF16, isOutput=False)
    lvwT = nc.declare_dram_parameter("lvwT", [KT, 128, H], BF16, isOutput=False)
    owT = nc.declare_dram_parameter("owT", [KT, 128, H], F32R, isOutput=False)
    lnw = nc.declare_dram_parameter("lnw", [H], F32, isOutput=False)
    lnb = nc.declare_dram_parameter("lnb", [H], F32, isOutput=False)
    ones_b = nc.declare_dram_parameter("ones_b", [128, 1], BF16, isOutput=False)
    selm = nc.declare_dram_parameter("selm", [33, 128], F32R, isOutput=False)

    if DBG:
        dbg_q = nc.declare_dram_parameter("dbg_q", [KT, 128, TP], F32, isOutput=True)
        dbg_k = nc.declare_dram_parameter("dbg_k", [KT, 128, TP], F32, isOutput=True)
        dbg_so = nc.declare_dram_parameter("dbg_so", [KT, 128, TP], F32, isOutput=True)
        dbg_bc = nc.declare_dram_parameter("dbg_bc", [KT, 128, TP], F32, isOutput=True)
        dbg_exp = nc.declare_dram_parameter("dbg_exp", [2, 128, 2, TP], mybir.dt.bfloat16, isOutput=True)
        dbg_et = nc.declare_dram_parameter("dbg_et", [1, 2, TP], mybir.dt.bfloat16, isOutput=True)
        dbg_v = nc.declare_dram_parameter("dbg_v", [2, 128, H], mybir.dt.bfloat16, isOutput=True)
    attn = nc.declare_dram_parameter("attn", [B, TOK, H], F32, isOutput=True)
    selfo = nc.declare_dram_parameter("selfo", [B, TOK, H], F32, isOutput=True)
    attn0 = nc.declare_dram_parameter("attn0", [B, H], F32, isOutput=True)
    self0 = nc.declare_dram_parameter("self0", [B, H], F32, isOutput=True)

    cc_in = nc.dram_tensor("cc_in", [SL, B, H], F32)
    cc_out = nc.dram_tensor("cc_out", [SEG, B, H], F32, addr_space="Shared")

    with TileContext(nc) as tc, \
         tc.tile_pool(name="const", bufs=1) as constp, \
         tc.tile_pool(name="wseg", bufs=1) as wsegp, \
         tc.tile_pool(name="work", bufs=1) as workp, \
         tc.tile_pool(name="pvp", bufs=2) as pvp, \
         tc.tile_pool(name="sop", bufs=1) as sop, \
         tc.tile_pool(name="lnp", bufs=2) as lnp, \
         tc.tile_pool(name="smp", bufs=4) as smp, \
         tc.tile_pool(name="ps", bufs=8, space="PSUM") as pp:

        # ---- constants -------------------------------------------------
        lnw_t = constp.tile([128, H], F32, tag="lnw", name="lnw")
        nc.sync.dma_start(out=lnw_t[:, :], in_=bass.AP(
            tensor=lnw, offset=0, ap=[[0, 128], [1, H]]))
        lnb_t = constp.tile([128, H], F32, tag="lnb", name="lnb")
        nc.sync.dma_start(out=lnb_t[:, :], in_=bass.AP(
            tensor=lnb, offset=0, ap=[[0, 128], [1, H]]))
        eps_t = constp.tile([128, 1], F32, tag="eps", name="eps")
        nc.vector.memset(eps_t[:, :], LN_EPS)
        onesb_t = constp.tile([128, 1], BF16, tag="onesb", name="onesb")
        nc.sync.dma_start(out=onesb_t[:, :], in_=ones_b[:, :])
        selm_t = constp.tile([33, 128], F32R, tag="selm", name="selm")
        nc.sync.dma_start(out=selm_t[:, :], in_=selm[:, :])
        blb_t = []
        for k in range(KT):
            t = constp.tile([128, 1], F32, tag=f"blb{k}", name=f"blb{k}")
            nc.sync.dma_start(out=t[:, :], in_=blb[k * 128:(k + 1) * 128])
            blb_t.append(t)
        owt = []
        for k in range(KT):
            t = constp.tile([128, H], F32R, tag=f"owt{k}", name=f"owt{k}")
            nc.sync.dma_start(out=t[:, :], in_=owT[k, :, :])
            owt.append(t)
        # catT groups (free dims [5, 8] = 40 cols): g0/g1 tails (sl=0/1, b),
        # g2 global lord (b), g3 segA lord (b), g4 segB lord (b)
        catT = [constp.tile([128, 5, 8], F32R, tag=f"cat{k}", name=f"cat{k}") for k in range(KT)]
        # ownT: (b, j): j=0 Blord, j=1 segA lord, j=2 segB lord (pre-attention)
        ownT = [constp.tile([128, 8, 3], BF16, tag=f"own{k}", name=f"own{k}") for k in range(KT)]

        def apply_ln(src, n):
            """LayerNorm over the free (H) axis of src[0:n, :]; returns tile."""
            stats = smp.tile([128, 2, 6], F32, tag="st", name="st")
            nc.vector.bn_stats(out=stats[0:n, 0, :], in_=src[0:n, 0:512])
            nc.vector.bn_stats(out=stats[0:n, 1, :], in_=src[0:n, 512:1024])
            mv = smp.tile([128, 2], F32, tag="mv", name="mv")
            nc.vector.bn_aggr(out=mv[0:n, :], in_=stats[0:n, :, :])
            rstd = smp.tile([128, 1], F32, tag="rstd", name="rstd")
            nc.scalar.activation(out=rstd[0:n, :], in_=mv[0:n, 1:2],
                                 func=AF.Sqrt, bias=eps_t[0:n, :])
            rstd2 = smp.tile([128, 1], F32, tag="rstd2", name="rstd2")
            nc.vector.reciprocal(out=rstd2[0:n, :], in_=rstd[0:n, :])
            nmr = smp.tile([128, 1], F32, tag="nmr", name="nmr")
            nc.vector.tensor_mul(nmr[0:n, :], mv[0:n, 0:1], rstd2[0:n, :])
            nc.vector.tensor_scalar_mul(nmr[0:n, :], nmr[0:n, :], -1.0)
            out_t = lnp.tile([128, H], F32, tag="rx", name="lno", bufs=3)
            nc.scalar.activation(out=out_t[0:n, :], in_=src[0:n, :],
                                 func=AF.Identity, bias=nmr[0:n, :],
                                 scale=rstd2[0:n, :])
            nc.vector.tensor_mul(out_t[0:n, :], out_t[0:n, :], lnw_t[0:n, :])
            nc.vector.tensor_add(out_t[0:n, :], out_t[0:n, :], lnb_t[0:n, :])
            return out_t

        # ---- phase A: per-segment QKV + local attention + projection ---
        for sl in range(SL):
            qw, kw, vw = [], [], []
            for k in range(KT):
                t = wsegp.tile([128, H], BF16, tag=f"qw{k}", name=f"qw{k}")
                nc.sync.dma_start(out=t[:, :], in_=qwT[sl, k, :, :])
                qw.append(t)
                t = wsegp.tile([128, H], BF16, tag=f"kw{k}", name=f"kw{k}")
                nc.sync.dma_start(out=t[:, :], in_=kwT[sl, k, :, :])
                kw.append(t)
                t = wsegp.tile([128, H], BF16, tag=f"vw{k}", name=f"vw{k}")
                nc.sync.dma_start(out=t[:, :], in_=vwT[sl, k, :, :])
                vw.append(t)

            for b in range(B):
                # -- load x^T slice for this (b, segment)
                xs = []
                for k in range(KT):
                    t = workp.tile([128, TP], BF16, tag=f"xs{k}", name=f"xs{k}", bufs=2)
                    nc.sync.dma_start(out=t[:, 0:T],
                                      in_=xT[b, k, :, sl * T:(sl + 1) * T])
                    nc.vector.memset(t[:, T:TP], 0.0)
                    xs.append(t)
                # -- Q^T, K^T: [H, T] stacked by o-tile
                qT, kT = [], []
                for o in range(KT):
                    psq = pp.tile([128, TP], F32, tag="ps", name="ps")
                    for i in range(KT):
                        nc.tensor.matmul(psq[:, :],
                                         qw[i][:, o * 128:(o + 1) * 128],
                                         xs[i][:, :],
                                         start=(i == 0), stop=(i == KT - 1))
                    t = workp.tile([128, TP], F32R, tag=f"qT{o}", name=f"qT{o}")
                    nc.vector.tensor_copy(out=t[:, :], in_=psq[:, :])
                    qT.append(t)
                    psk = pp.tile([128, TP], F32, tag="ps", name="ps")
                    for i in range(KT):
                        nc.tensor.matmul(psk[:, :],
                                         kw[i][:, o * 128:(o + 1) * 128],
                                         xs[i][:, :],
                                         start=(i == 0), stop=(i == KT - 1))
                    t = workp.tile([128, TP], F32R, tag=f"kT{o}", name=f"kT{o}")
                    nc.vector.tensor_copy(out=t[:, :], in_=psk[:, :])
                    kT.append(t)
                # -- V natural [T, H] in bf16 (tiles of 128/128/1 rows)
                vn = [pvp.tile([128, H], BF16, tag="vn0", name="vn0"),
                      pvp.tile([128, H], BF16, tag="vn1", name="vn1"),
                      pvp.tile([1, H], BF16, tag="vn2", name="vn2", bufs=1)]
                for tt, (t0, tsz) in enumerate(TKT):
                    for nch in range(2):
                        psv = pp.tile([128, 512], F32, tag="ps", name="ps")
                        for i in range(KT):
                            nc.tensor.matmul(psv[0:tsz, :],
                                             xs[i][:, t0:t0 + tsz],
                                             vw[i][:, nch * 512:(nch + 1) * 512],
                                             start=(i == 0), stop=(i == KT - 1))
                        nc.vector.tensor_copy(
                            out=vn[tt][0:tsz, nch * 512:(nch + 1) * 512],
                            in_=psv[0:tsz, :])

                if DBG and sl == 0 and b == 0:
                    for o in range(KT):
                        nc.sync.dma_start(out=dbg_q[o, :, :], in_=qT[o][:, :].bitcast(F32))
                        nc.sync.dma_start(out=dbg_k[o, :, :], in_=kT[o][:, :].bitcast(F32))
                    nc.sync.dma_start(out=dbg_v[0, :, :], in_=vn[0][:, :])
                    nc.sync.dma_start(out=dbg_v[1, :, :], in_=vn[1][:, :])
                # -- attention per head pair (heads 2hp, 2hp+1)
                soT = [sop.tile([128, TP], F32R, tag=f"soT{o}", name=f"soT{o}") for o in range(KT)]
                for hp in range(KT):
                    exps = [pvp.tile([128, 2, TP], BF16, tag="expA", name="expA"),
                            pvp.tile([128, 2, TP], BF16, tag="expB", name="expB")]
                    etail = pvp.tile([1, 2, TP], BF16, tag="expT", name="expT")
                    pssum = pp.tile([33, TP], F32, tag="ps", name="ps")
                    for h2 in range(2):
                        hb = h2 * 64
                        for tkt, (t0, tsz) in enumerate(TKT):
                            pss = pp.tile([128, TP], F32, tag="ps", name="ps")
                            nc.tensor.matmul(pss[0:tsz, :],
                                             kT[hp][hb:hb + 64, t0:t0 + tsz],
                                             qT[hp][hb:hb + 64, :],
                                             start=True, stop=True)
                            if tkt < 2:
                                nc.scalar.activation(
                                    out=exps[h2][0:tsz, tkt, :],
                                    in_=pss[0:tsz, :], func=AF.Exp, scale=SCALE)
                            else:
                                nc.scalar.activation(
                                    out=etail[0:1, h2, :],
                                    in_=pss[0:1, :], func=AF.Exp, scale=SCALE)
                        for tkt, (t0, tsz) in enumerate(TKT):
                            rhs = (exps[h2][0:tsz, tkt, :] if tkt < 2
                                   else etail[0:1, h2, :])
                            nc.tensor.matmul(
                                pssum[h2 * 32:h2 * 32 + 1, :],
                                onesb_t[0:tsz, :], rhs,
                                start=(tkt == 0), stop=(tkt == 2),
                                tile_position=(0, h2 * 32))
                    pspv = pp.tile([128, TP], F32, tag="ps", name="ps")
                    for h2 in range(2):
                        hb = h2 * 64
                        for tkt, (t0, tsz) in enumerate(TKT):
                            rhs = (exps[h2][0:tsz, tkt, :] if tkt < 2
                                   else etail[0:1, h2, :])
                            nc.tensor.matmul(
                                pspv[hb:hb + 64, :],
                                vn[tkt][0:tsz,
                                        hp * 128 + hb:hp * 128 + hb + 64],
                                rhs,
                                start=(tkt == 0), stop=(tkt == 2),
                                tile_position=(0, hb))
                    rec = smp.tile([33, TP], F32R, tag="rec", name="rec")
                    with nc.allow_low_precision(reason="f32r recip, 23->19 bits"):
                        nc.vector.reciprocal(out=rec[0:1, :], in_=pssum[0:1, :])
                        nc.vector.reciprocal(out=rec[32:33, :],
                                             in_=pssum[32:33, :])
                    bcp = pp.tile([128, TP], F32, tag="ps", name="ps")
                    nc.tensor.matmul(bcp[:, :], selm_t[0:1, :], rec[0:1, :],
                                     start=True, stop=False)
                    nc.tensor.matmul(bcp[:, :], selm_t[32:33, :], rec[32:33, :],
                                     start=False, stop=True)
                    bc = lnp.tile([128, TP], F32, tag="bc", name="bc")
                    nc.vector.tensor_copy(out=bc[:, :], in_=bcp[:, :])
                    nc.vector.tensor_mul(soT[hp][:, :], pspv[:, :], bc[:, :])
                    if DBG and sl == 0 and b == 0:
                        nc.sync.dma_start(out=dbg_so[hp, :, :], in_=soT[hp][:, :].bitcast(F32))
                        nc.sync.dma_start(out=dbg_bc[hp, :, :], in_=bc[:, :])
                        if hp == 0:
                            nc.sync.dma_start(out=dbg_exp[0, :, :, :], in_=exps[0][:, :, :])
                            nc.sync.dma_start(out=dbg_exp[1, :, :, :], in_=exps[1][:, :, :])
                            nc.sync.dma_start(out=dbg_et[0, :, :], in_=etail[0:1, :, :])
                    # lord column -> own queries + collective input
                    nc.vector.tensor_copy(
                        out=ownT[hp][:, b, 1 + sl:2 + sl],
                        in_=soT[hp][:, 0:1])
                    nc.sync.dma_start(out=cc_in[sl, b, hp * 128:(hp + 1) * 128],
                                      in_=soT[hp][:, 0:1].bitcast(F32))
                    # tail column (token 256) for the deferred projection
                    nc.vector.tensor_copy(
                        out=catT[hp][:, sl, b:b + 1],
                        in_=soT[hp][:, 256:257])

                # -- bulk self_outputs store (transposed), skip lord col 0
                for k in range(KT):
                    dst = bass.AP(tensor=selfo,
                                  offset=(b * TOK + sl * T + 1) * H + k * 128,
                                  ap=[[1, 128], [H, 256]])
                    nc.sync.dma_start(out=dst, in_=soT[k][:, 1:T].bitcast(F32))

                # -- immediate out-projection + residual + LN for t-tiles 0,1
                for tt2 in range(2):
                    rx = lnp.tile([128, H], F32, tag="rx", name="rx", bufs=3)
                    nc.sync.dma_start(
                        out=rx[:, :],
                        in_=xnat[b, sl * T + tt2 * 128:sl * T + (tt2 + 1) * 128, :])
                    hh = lnp.tile([128, H], F32, tag="hh", name="hh")
                    for nch in range(2):
                        psp = pp.tile([128, 512], F32, tag="ps", name="ps")
                        for k in range(KT):
                            nc.tensor.matmul(psp[:, :],
                                             soT[k][:, tt2 * 128:(tt2 + 1) * 128],
                                             owt[k][:, nch * 512:(nch + 1) * 512],
                                             start=(k == 0), stop=(k == KT - 1))
                        nc.vector.tensor_add(hh[:, nch * 512:(nch + 1) * 512],
                                             psp[:, :],
                                             rx[:, nch * 512:(nch + 1) * 512])
                    lno = apply_ln(hh, 128)
                    if tt2 == 0:
                        nc.sync.dma_start(
                            out=attn[b, sl * T + 1:sl * T + 128, :],
                            in_=lno[1:128, :])
                    else:
                        nc.sync.dma_start(
                            out=attn[b, sl * T + 128:sl * T + 256, :],
                            in_=lno[:, :])

        # ---- phase B: global lord attention ----------------------------
        # B0: Blord^T = blwT.T @ h0^T + blb  -> ownT[:, :, 0]
        h0t = []
        for k in range(KT):
            t = smp.tile([128, B], BF16, tag=f"h0t{k}", name=f"h0t{k}", bufs=1)
            nc.sync.dma_start(out=t[:, :], in_=h0T[k, :, :])
            h0t.append(t)
        blw = []
        for k in range(KT):
            t = wsegp.tile([128, H], BF16, tag=f"qw{k}", name=f"qw{k}")
            nc.sync.dma_start(out=t[:, :], in_=blwT[k, :, :])
            blw.append(t)
        for o in range(KT):
            psb = pp.tile([128, B], F32, tag="ps", name="ps")
            for i in range(KT):
                nc.tensor.matmul(psb[:, :], blw[i][:, o * 128:(o + 1) * 128],
                                 h0t[i][:, :], start=(i == 0), stop=(i == KT - 1))
            nc.scalar.activation(out=ownT[o][:, :, 0], in_=psb[:, :],
                                 func=AF.Identity, bias=blb_t[o][:, :], scale=1.0)

        # B1: all-gather the segment lords
        nc.gpsimd.collective_compute(
            "AllGather", ALU.bypass,
            replica_groups=[list(range(N_CORES))],
            ins=[cc_in[:, :, :]], outs=[cc_out[:, :, :]])

        # B2: lords_in^T tiles [128][dup 2, b 8, t 17]; t=0 Blord, t=1+s
        # gathered lords; the second copy (dup=1) pads the moving dim to 272.
        lT = []
        for k in range(KT):
            t = sop.tile([128, 2, 8, 17], BF16, tag=f"soT{k}", name=f"lT{k}")
            lT.append(t)
            for bb in range(B):
                srcb = bass.AP(tensor=cc_out, offset=k * 128 + bb * H,
                               ap=[[1, 128], [B * H, 16]])
                nc.gpsimd.dma_start(out=t[:, 0, bb, 1:17], in_=srcb)
            nc.vector.tensor_copy(out=t[:, 0, :, 0:1],
                                  in_=ownT[k][:, :, 0:1])
            nc.vector.tensor_copy(out=t[:, 1, :, :], in_=t[:, 0, :, :])

        # B3: lord projections
        lqw, lkw, lvw = [], [], []
        for k in range(KT):
            t = wsegp.tile([128, H], BF16, tag=f"kw{k}", name=f"kw{k}")
            nc.sync.dma_start(out=t[:, :], in_=lqwT[k, :, :])
            lqw.append(t)
            t = wsegp.tile([128, H], BF16, tag=f"vw{k}", name=f"vw{k}")
            nc.sync.dma_start(out=t[:, :], in_=lkwT[k, :, :])
            lkw.append(t)
            t = wsegp.tile([128, H], BF16, tag=f"lvw{k}", name=f"lvw{k}")
            nc.sync.dma_start(out=t[:, :], in_=lvwT[k, :, :])
            lvw.append(t)
        lqo, lk = [], []
        for o in range(KT):
            psq = pp.tile([128, 24], F32, tag="ps", name="ps")
            for i in range(KT):
                nc.tensor.matmul(psq[:, :], lqw[i][:, o * 128:(o + 1) * 128],
                                 ownT[i][:, :, :], start=(i == 0),
                                 stop=(i == KT - 1))
            t = smp.tile([128, 8, 3], BF16, tag=f"lqo{o}", name=f"lqo{o}", bufs=1)
            nc.vector.tensor_copy(out=t[:, :, :], in_=psq[:, :])
            lqo.append(t)
            psk = pp.tile([128, 272], F32, tag="ps", name="ps")
            for i in range(KT):
                nc.tensor.matmul(psk[:, :], lkw[i][:, o * 128:(o + 1) * 128],
                                 lT[i][:, :, :, :], start=(i == 0),
                                 stop=(i == KT - 1))
            t = smp.tile([128, 8, 17], BF16, tag=f"lk{o}", name=f"lk{o}", bufs=1)
            nc.vector.tensor_copy(out=t[:, :, :], in_=psk[:, 0:136])
            lk.append(t)
        # lv natural, rows padded: row (b%4)*32 + t within group g = b//4
        lvn = []
        for g in range(2):
            t = pvp.tile([128, H], BF16, tag=f"vn{g}", name=f"lvn{g}")
            for nch in range(2):
                psv = pp.tile([128, 512], F32, tag="ps", name="ps")
                for bi in range(4):
                    bb = g * 4 + bi
                    rb = bi * 32
                    for i in range(KT):
                        nc.tensor.matmul(psv[rb:rb + 17, :],
                                         lT[i][:, 0, bb, :],
                                         lvw[i][:, nch * 512:(nch + 1) * 512],
                                         start=(i == 0), stop=(i == KT - 1),
                                         tile_position=(0, rb))
                    nc.vector.tensor_copy(
                        out=t[rb:rb + 17, nch * 512:(nch + 1) * 512],
                        in_=psv[rb:rb + 17, :])
            lvn.append(t)

        # B5: lord attention per (b, head); queries = [global, segA, segB]
        for b in range(B):
            g, rb = b // 4, (b % 4) * 32
            for hp in range(KT):
                pslo = pp.tile([128, 3], F32, tag="ps", name="ps")
                for h2 in range(2):
                    hb = h2 * 64
                    h = hp * 2 + h2
                    psls = pp.tile([128, 3], F32, tag="ps", name="ps")
                    nc.tensor.matmul(psls[rb:rb + 17, :],
                                     lk[hp][hb:hb + 64, b, :],
                                     lqo[hp][hb:hb + 64, b, :],
                                     start=True, stop=True,
                                     tile_position=(hb, rb))
                    els = smp.tile([128, 3], BF16, tag="els", name="els")
                    nc.scalar.activation(out=els[rb:rb + 17, :],
                                         in_=psls[rb:rb + 17, :],
                                         func=AF.Exp, scale=SCALE)
                    psls2 = pp.tile([33, 3], F32, tag="ps", name="ps")
                    nc.tensor.matmul(psls2[h2 * 32:h2 * 32 + 1, :],
                                     onesb_t[rb:rb + 17, :],
                                     els[rb:rb + 17, :], start=True, stop=True,
                                     tile_position=(rb, h2 * 32))
                    rls = smp.tile([33, 4], F32R, tag="rls", name="rls")
                    with nc.allow_low_precision(reason="f32r recip"):
                        nc.vector.reciprocal(out=rls[h2 * 32:h2 * 32 + 1, 0:3],
                                             in_=psls2[h2 * 32:h2 * 32 + 1, :])
                    bclp = pp.tile([128, 4], F32, tag="ps", name="ps")
                    nc.tensor.matmul(bclp[:, :],
                                     selm_t[h2 * 32:h2 * 32 + 1, :],
                                     rls[h2 * 32:h2 * 32 + 1, :],
                                     start=True, stop=True)
                    bcl = smp.tile([128, 4], F32, tag="bcl", name="bcl")
                    nc.vector.tensor_copy(out=bcl[:, :], in_=bclp[:, :])
                    nc.tensor.matmul(pslo[hb:hb + 64, :],
                                     lvn[g][rb:rb + 17, h * 64:(h + 1) * 64],
                                     els[rb:rb + 17, :], start=True, stop=True,
                                     tile_position=(rb, hb))
                    nc.vector.tensor_mul(catT[hp][hb:hb + 64, 2:5, b],
                                         pslo[hb:hb + 64, :],
                                         bcl[hb:hb + 64, 0:3])

        # B6: store the lord self_outputs rows straight from catT columns
        for k in range(KT):
            nc.sync.dma_start(
                out=bass.AP(tensor=self0, offset=k * 128,
                            ap=[[1, 128], [H, 8]]),
                in_=catT[k][:, 2, :].bitcast(F32))
            for sl2 in range(SL):
                nc.sync.dma_start(
                    out=bass.AP(tensor=selfo, offset=sl2 * T * H + k * 128,
                                ap=[[1, 128], [TOK * H, 8]]),
                    in_=catT[k][:, 3 + sl2, :].bitcast(F32))

        # B7: final batched projection of [tails(16), global(8), lords(16)]
        rf = lnp.tile([128, H], F32, tag="rx", name="rf", bufs=3)
        nc.sync.dma_start(out=rf[0:16, :], in_=bass.AP(
            tensor=xnat, offset=256 * H,
            ap=[[T * H, 2], [TOK * H, 8], [1, H]]))
        nc.sync.dma_start(out=rf[16:24, :], in_=h0nat[:, :])
        nc.sync.dma_start(out=rf[24:40, :], in_=bass.AP(
            tensor=xnat, offset=0,
            ap=[[T * H, 2], [TOK * H, 8], [1, H]]))
        hf = lnp.tile([128, H], F32, tag="hh", name="hh")
        for nch in range(2):
            psf = pp.tile([128, 512], F32, tag="ps", name="ps")
            for k in range(KT):
                nc.tensor.matmul(psf[0:40, :], catT[k][:, :, :],
                                 owt[k][:, nch * 512:(nch + 1) * 512],
                                 start=(k == 0), stop=(k == KT - 1))
            nc.vector.tensor_add(hf[0:40, nch * 512:(nch + 1) * 512],
                                 psf[0:40, :],
                                 rf[0:40, nch * 512:(nch + 1) * 512])
        lnf = apply_ln(hf, 40)
        nc.sync.dma_start(out=bass.AP(
            tensor=attn, offset=256 * H,
            ap=[[T * H, 2], [TOK * H, 8], [1, H]]), in_=lnf[0:16, :])
        nc.sync.dma_start(out=attn0[:, :], in_=lnf[16:24, :])
        nc.sync.dma_start(out=bass.AP(
            tensor=attn, offset=0,
            ap=[[T * H, 2], [TOK * H, 8], [1, H]]), in_=lnf[24:40, :])

    nc.finalize()
    return nc


_PROGRAM = None


def _get_program():
    global _PROGRAM
    if _PROGRAM is None:
        _PROGRAM = build_program()
    return _PROGRAM


def _prep_in_maps(inputs):
    import ml_dtypes
    hs = np.asarray(inputs["hidden_states"], np.float32)
    seg_qw = np.asarray(inputs["seg_qw"], np.float32)
    seg_kw = np.asarray(inputs["seg_kw"], np.float32)
    seg_vw = np.asarray(inputs["seg_vw"], np.float32)
    out_b = np.asarray(inputs["out_b"], np.float32)

    h0 = hs[:, 0, :]                                   # [B, H]
    h0T = np.ascontiguousarray(h0.T).reshape(KT, 128, B).astype(ml_dtypes.bfloat16)
    h0nat = np.ascontiguousarray(h0 + out_b[None, :])
    blwT = np.ascontiguousarray(np.asarray(inputs["Blord_w"], np.float32).T
                                ).reshape(KT, 128, H).astype(ml_dtypes.bfloat16)
    lqwT = np.ascontiguousarray(np.asarray(inputs["lord_qw"], np.float32).T
                                ).reshape(KT, 128, H).astype(ml_dtypes.bfloat16)
    lkwT = np.ascontiguousarray(np.asarray(inputs["lord_kw"], np.float32).T
                                ).reshape(KT, 128, H).astype(ml_dtypes.bfloat16)
    lvwT = np.ascontiguousarray(np.asarray(inputs["lord_vw"], np.float32).T
                                ).reshape(KT, 128, H).astype(ml_dtypes.bfloat16)
    owTt = np.ascontiguousarray(np.asarray(inputs["out_w"], np.float32).T
                                ).reshape(KT, 128, H)
    blb = np.asarray(inputs["Blord_b"], np.float32)
    lnw = np.asarray(inputs["ln_w"], np.float32)
    lnb = np.asarray(inputs["ln_b"], np.float32)
    ones_bf = np.ones((128, 1), ml_dtypes.bfloat16)
    selm_np = np.zeros((33, 128), np.float32)
    selm_np[0, 0:64] = 1.0
    selm_np[32, 64:128] = 1.0

    in_maps = []
    for c in range(N_CORES):
        tok0 = 1 + c * TOK
        xsl = hs[:, tok0:tok0 + TOK, :]                # [B, 514, H]
        xTc = np.ascontiguousarray(xsl.transpose(0, 2, 1)).reshape(
            B, KT, 128, TOK).astype(ml_dtypes.bfloat16)
        xnat = np.ascontiguousarray(xsl + out_b[None, None, :])
        qwTc = np.ascontiguousarray(
            seg_qw[2 * c:2 * c + 2].transpose(0, 2, 1)).reshape(SL, KT, 128, H).astype(ml_dtypes.bfloat16)
        kwTc = np.ascontiguousarray(
            seg_kw[2 * c:2 * c + 2].transpose(0, 2, 1)).reshape(SL, KT, 128, H).astype(ml_dtypes.bfloat16)
        vwTc = np.ascontiguousarray(
            seg_vw[2 * c:2 * c + 2].transpose(0, 2, 1)).reshape(SL, KT, 128, H).astype(ml_dtypes.bfloat16)
        in_maps.append({
            "xT": xTc, "xnat": xnat,
            "h0T": h0T, "h0nat": h0nat,
            "qwT": qwTc, "kwT": kwTc, "vwT": vwTc,
            "blwT": blwT, "blb": blb,
            "lqwT": lqwT, "lkwT": lkwT, "lvwT": lvwT,
            "owT": owTt, "lnw": lnw, "lnb": lnb,
            "ones_b": ones_bf, "selm": selm_np,
        })
    return in_maps


def _run(inputs, trace=False):
    from concourse.bass_utils import run_bass_kernel_spmd
    nc = _get_program()
    in_maps = _prep_in_maps(inputs)
    res = run_bass_kernel_spmd(nc, in_maps, list(range(N_CORES)), trace=trace)
    attention_output = np.empty((B, L, H), np.float32)
    self_outputs = np.empty((B, L, H), np.float32)
    attention_output[:, 0, :] = res.results[0]["attn0"]
    self_outputs[:, 0, :] = res.results[0]["self0"]
    for c in range(N_CORES):
        tok0 = 1 + c * TOK
        attention_output[:, tok0:tok0 + TOK, :] = res.results[c]["attn"]
        self_outputs[:, tok0:tok0 + TOK, :] = res.results[c]["selfo"]
    return (attention_output, self_outputs), res


def kernel(**inputs):
    (attention_output, self_outputs), _ = _run(inputs, trace=False)
    return attention_output, self_outputs

